# revision 1
# baseline (speedup 1.0000x reference)
"""CKConv (nn_CKConv_85950885527678) Trainium2 Bass kernel.

Strategy: data-parallel over batch (8 batches -> 8 NeuronCores). The tiny
SIREN kernel network (~134 MFLOP) is evaluated on the host and the generated
conv kernel is replicated to every core in a matmul-ready layout (as the
sharding hint suggests: "replicate the tiny SIREN params and generated
kernel").

Per core the causal conv out[o,t] = sum_{i,l} K[o,i,l] * xpad[i,t+l]
(xpad = x left-padded with T zeros, taps l in [1,2048]; l=0 never
contributes) is computed as a block-triangular matmul:
  - taps grouped into 512 blocks of 4; contraction K = 128 = (4 taps x 32 in
    channels) per matmul
  - W[dd*32+i, blk*32+o] = K[o,i,4*blk+1+dd]  (stationary operand)
  - XP[dd*32+i, c] = xpad[i, U0+c+dd]         (moving operand; a single
    shifted-replicated copy serves every tap block via column offsets)
  - 4 tap blocks run concurrently in the four 32-wide PE column groups
    (tile_position), accumulating in 4 psum partition ranges; a constant
    S = tile(I_32, 4x1) matmul reduces the groups, then bias is added.
"""

import os
import numpy as np

B, C_IN, C_OUT, T, D = 8, 32, 32, 2048, 32
L = T + 1
NBLK = 512
U0 = 1534
XPW = 2565
NT = 4
N_CORES = 8

_cache = {}


# ---------------------------------------------------------------- host prep

def _siren_kernel(pos_rel, w1, b1, w2, b2, w3, b3):
    p = pos_rel.reshape(1, L).astype(np.float32)
    h = np.sin(w1.astype(np.float32) @ p + b1[:, None].astype(np.float32))
    h = np.sin(w2.astype(np.float32) @ h + b2[:, None].astype(np.float32))
    k = w3.astype(np.float32) @ h + b3[:, None].astype(np.float32)
    return k.astype(np.float32)


def _build_w(k):
    kk = k.reshape(C_OUT, C_IN, L)[:, :, 1:]
    arr = kk.reshape(C_OUT, C_IN, NBLK, 4)
    return np.ascontiguousarray(
        arr.transpose(3, 1, 2, 0).reshape(128, NBLK * C_OUT)
    ).astype(np.float32)


def _build_xp(x):
    xpad = np.zeros((B, C_IN, 2 * T + 8), np.float32)
    xpad[:, :, T : 2 * T] = x
    XP = np.empty((B, 128, XPW), np.float32)
    for dd in range(4):
        XP[:, dd * 32 : (dd + 1) * 32, :] = xpad[:, :, U0 + dd : U0 + dd + XPW]
    return XP


# ------------------------------------------------------- tile drain patch

def _patch_tile_drain():
    """This walrus build rejects >2 sync waits on a CTRL (Drain) instruction;
    spread the TileContext exit waits over single-wait NOPs instead."""
    from concourse.tile import TileContext
    from concourse.vector_clock import ScopedClock, VectorClock

    if getattr(TileContext, "_ck_drain_patched", False):
        return

    def _drain_and_barrier(self, tick_clock, wait_clock):
        gc = tick_clock.global_clock
        n = len(gc)
        for p in range(n):
            if gc[p] <= 0:
                continue
            vec = [gc[q] if q == p else 0 for q in range(n)]
            nop = self.nc.sync.nop(nofuse=True, hint=f"split_drain_wait_p{p}")
            wait_clock.add_sem_waits(nop.ins, ScopedClock({None: VectorClock(vec)}))
        self.nc.sync.drain()
        self.nc.all_engine_barrier()
        assert self.sems is not None
        popped = self.nc._tile_sem_poison_stack.pop()
        assert popped is self._sem_poison
        self.nc.clear_and_free_semaphores(list(self.sems.allocated().values()))
        self.nc.all_engine_barrier()

    TileContext._drain_and_barrier = _drain_and_barrier
    TileContext._ck_drain_patched = True


WAIT_LIMIT = 1  # this walrus build encodes at most 2 sync waits per instruction


def _split_excess_waits(nc, limit=WAIT_LIMIT):
    """Hoist excess sem waits onto same-engine NOPs placed just before the
    instruction — in-order engine queues make this semantically identical."""
    import concourse.mybir as mybir

    n_split = 0
    for f in nc.m.functions:
        for bb in f.blocks:
            new_insts = []
            changed = False
            for inst in bb.instructions:
                si = inst.sync_info
                waits = list(si.on_wait) if si is not None and si.on_wait else []
                if len(waits) > limit:
                    extra, keep = waits[:-limit], waits[-limit:]
                    for i in range(0, len(extra), limit):
                        n_split += 1
                        new_insts.append(
                            mybir.InstNoOp(
                                name=f"I-ckwsplit-{n_split}",
                                engine=inst.engine,
                                ins=[],
                                outs=[],
                                sync_info=mybir.SyncInfo(
                                    on_wait=extra[i : i + limit], on_update=[]
                                ),
                            )
                        )
                    inst.sync_info = mybir.SyncInfo(
                        on_wait=keep, on_update=list(si.on_update) if si.on_update else []
                    )
                    changed = True
                new_insts.append(inst)
            if changed:
                bb.instructions = new_insts
    return n_split


# ------------------------------------------------------------ device kernel

def _build_nc(mm_dtype_name):
    import concourse.bass as bass
    import concourse.mybir as mybir
    from concourse.tile import TileContext

    _patch_tile_drain()
    f32 = mybir.dt.float32
    mm_dt = getattr(mybir.dt, mm_dtype_name)

    nc = bass.Bass()
    xp_d = nc.declare_dram_parameter("xp", [128, XPW], mm_dt, isOutput=False)
    w_d = nc.declare_dram_parameter("w", [128, NBLK * 32], mm_dt, isOutput=False)
    s_d = nc.declare_dram_parameter("s", [128, 32], f32, isOutput=False)
    bias_d = nc.declare_dram_parameter("bias", [32, 1], f32, isOutput=False)
    out_d = nc.declare_dram_parameter("out", [32, T], f32, isOutput=True)

    # float32r matmuls reject tile_position on this walrus -> untiled path
    tiled = mm_dtype_name != "float32r"

    with TileContext(nc) as tc:
        with (
            tc.tile_pool(name="const", bufs=1) as const,
            tc.tile_pool(name="work", bufs=3) as work,
            tc.tile_pool(name="acc_psum", bufs=4, space="PSUM") as acc_psum,
            tc.tile_pool(name="red_psum", bufs=2, space="PSUM") as red_psum,
        ):
            xp_sb = const.tile([128, XPW], mm_dt)
            nc.sync.dma_start(xp_sb[:, :], xp_d[:, :])
            s_sb = const.tile([128, 32], f32)
            nc.sync.dma_start(s_sb[:, :], s_d[:, :])
            bias_sb = const.tile([32, 1], f32)
            nc.sync.dma_start(bias_sb[:, :], bias_d[:, :])
            w_sb = const.tile([128, NBLK * 32], mm_dt)
            # chunk order matches first use: tile j=0 needs blk>=384 first
            for lo, hi in [(384, 512), (256, 384), (128, 256), (0, 128)]:
                nc.sync.dma_start(w_sb[:, lo * 32 : hi * 32], w_d[:, lo * 32 : hi * 32])

            max_tiles = int(os.environ.get("CK_MAX_TILES", str(NT)))
            max_rounds = int(os.environ.get("CK_MAX_ROUNDS", "99999"))
            for j in range(max_tiles):
                t0 = 512 * j
                blo = 128 * (3 - j)
                rounds = list(range(blo, NBLK, 4))[:max_rounds]
                last_r = len(rounds) - 1
                if tiled:
                    acc = acc_psum.tile([128, 512], f32)
                    for r, blk0 in enumerate(rounds):
                        for g in range(4):
                            blk = blk0 + g
                            off = t0 + 4 * blk + 1 - U0
                            nc.tensor.matmul(
                                acc[32 * g : 32 * (g + 1), :],
                                w_sb[:, 32 * blk : 32 * blk + 32],
                                xp_sb[:, off : off + 512],
                                start=(r == 0),
                                stop=(r == last_r),
                                tile_position=(0, 32 * g),
                            )
                    red = work.tile([128, 512], f32)
                    nc.vector.tensor_copy(red[:, :], acc[:, :])
                    ps2 = red_psum.tile([32, 512], f32)
                    nc.tensor.matmul(ps2[:, :], s_sb[:, :], red[:, :],
                                     start=True, stop=True)
                    ot = work.tile([32, 512], f32)
                    nc.vector.tensor_scalar_add(ot[:, :], ps2[:, :], bias_sb[:, :])
                else:
                    acc = acc_psum.tile([32, 512], f32)
                    blks = [b for r0 in rounds for b in range(r0, r0 + 4)]
                    nblks = len(blks)
                    for r, blk in enumerate(blks):
                        off = t0 + 4 * blk + 1 - U0
                        nc.tensor.matmul(
                            acc[:, :],
                            w_sb[:, 32 * blk : 32 * blk + 32],
                            xp_sb[:, off : off + 512],
                            start=(r == 0),
                            stop=(r == nblks - 1),
                        )
                    ot = work.tile([32, 512], f32)
                    nc.vector.tensor_scalar_add(ot[:, :], acc[:, :], bias_sb[:, :])
                nc.sync.dma_start(out_d[:, t0 : t0 + 512], ot[:, :])
    _split_excess_waits(nc)
    return nc


# ------------------------------------------------------------------- entry

def kernel(**inputs):
    from concourse.bass_utils import run_bass_kernel_spmd

    x = np.asarray(inputs["x"], dtype=np.float32)
    k = _siren_kernel(
        np.asarray(inputs["pos_rel"]), np.asarray(inputs["w1"]),
        np.asarray(inputs["b1"]), np.asarray(inputs["w2"]),
        np.asarray(inputs["b2"]), np.asarray(inputs["w3"]),
        np.asarray(inputs["b3"]),
    )
    W = _build_w(k)
    XP = _build_xp(x)
    S = np.tile(np.eye(C_OUT, dtype=np.float32), (4, 1))
    bias = np.ascontiguousarray(
        np.asarray(inputs["bias"], dtype=np.float32).reshape(32, 1)
    )

    mm_dtype = os.environ.get("CK_MM_DTYPE", "bfloat16")
    if "nc" not in _cache or _cache.get("mm_dtype") != mm_dtype:
        _cache["nc"] = _build_nc(mm_dtype)
        _cache["mm_dtype"] = mm_dtype
    nc = _cache["nc"]

    if mm_dtype == "bfloat16":
        import ml_dtypes

        W = W.astype(ml_dtypes.bfloat16)
        XP = XP.astype(ml_dtypes.bfloat16)

    n_cores = int(os.environ.get("CK_CORES", str(N_CORES)))
    in_maps = [
        {"xp": XP[b % B], "w": W, "s": S, "bias": bias} for b in range(n_cores)
    ]

    # The axon-tunneled device occasionally throws a transient
    # NRT_EXEC_UNIT_UNRECOVERABLE on 8-core launches; retry, then fall back
    # to two 4-core waves (same NEFF, batches split across waves).
    res = None
    for attempt in range(3):
        try:
            res = run_bass_kernel_spmd(nc, in_maps, core_ids=list(range(n_cores)))
            break
        except Exception:
            if attempt == 2:
                res = None
            else:
                continue
    if res is not None:
        out = np.stack(
            [res.results[b % n_cores]["out"] for b in range(B)], axis=0
        )
        return out.astype(np.float32)

    half = n_cores // 2 if n_cores > 1 else 1
    outs = []
    for w0 in range(0, B, half):
        wave_maps = [
            {"xp": XP[(w0 + c) % B], "w": W, "s": S, "bias": bias}
            for c in range(half)
        ]
        wres = run_bass_kernel_spmd(nc, wave_maps, core_ids=list(range(half)))
        outs.extend(wres.results[c]["out"] for c in range(half))
    out = np.stack(outs[:B], axis=0)
    return out.astype(np.float32)



# revision 7
# speedup vs baseline: 4.6390x; 4.6390x over previous
"""CKConv (nn_CKConv_85950885527678) Trainium2 Bass kernel.

Strategy: data-parallel over batch (8 batches -> 8 NeuronCores). The tiny
SIREN kernel network is evaluated on the host; the generated conv kernel is
replicated to every core (per the sharding hint).

Per core the causal conv out[o,t] = sum_{i,l} K[o,i,l] * xp[i,t+l]
(xp = x left-padded with 2048 zeros; taps l in [1,2048]) is computed with
full-width M=128 matmuls via time-decimation by 4:

  - output element (r,o) at psum column c holds out[o, 4c+r]
    (r in 0..3 output-time residue, o in 0..31 out channel)
  - moving operand XPb[(dd,i), u] = xp[i, 4u+dd] - a stride-4 view whose
    column shift by m advances all taps by 4m
  - stationary block m (m = 512..1): W_m[(dd,i),(r,o)] = K[o,i,4m+dd-r]
    (zero out of range); psum accumulates over m
  - causality: block m contributes only to output cols c >= 512-m (earlier
    cols read the zero left-pad), so each block issues variable-width
    matmuls covering exactly [512-m, 512), split at psum-group boundaries
    (4 psum banks of 128 cols, drained + bias-added as they complete)

Every matmul is M=128 x K=128, so the PE array is fully used and the charged
moving columns hit the causal lower bound (~131k col-cycles). The weight
blocks are streamed (16.8 MB bf16) in block order matching first use so the
DMA hides under the matmul stream.
"""

import os
import numpy as np

B, C_IN, C_OUT, T, D = 8, 32, 32, 2048, 32
L = T + 1
NBLK = 512          # tap blocks m = 512..1 (block index b = 512-m)
NGROUPS = 4         # psum accumulator groups (one bank each, 128 cols)
GW = 128            # cols per psum group
XPW = 1024          # XPb columns (u = c+m in [1, 1023])
W_COLS = NBLK * 128
N_CORES = 8
W_CHUNK_BLKS = 8    # steady-state W DMA chunk: 8 blocks = 256 KB

_cache = {}


# ---------------------------------------------------------------- host prep

def _siren_kernel(pos_rel, w1, b1, w2, b2, w3, b3):
    p = pos_rel.reshape(1, L).astype(np.float32)
    h = np.sin(w1.astype(np.float32) @ p + b1[:, None].astype(np.float32))
    h = np.sin(w2.astype(np.float32) @ h + b2[:, None].astype(np.float32))
    k = w3.astype(np.float32) @ h + b3[:, None].astype(np.float32)
    return k.reshape(C_OUT, C_IN, L).astype(np.float32)


def _build_w(K):
    """W[dd*32+i, b*128 + r*32+o] = K[o,i,4m+dd-r], m = 512-b."""
    ms = np.arange(NBLK, 0, -1)
    W6 = np.zeros((4, C_IN, NBLK, 4, C_OUT), np.float32)
    for dd in range(4):
        for r in range(4):
            l = 4 * ms + dd - r
            valid = (l >= 1) & (l <= 2048)
            src = K[:, :, np.clip(l, 0, 2048)]          # (o, i, b)
            src = np.where(valid[None, None, :], src, 0.0)
            W6[dd, :, :, r, :] = src.transpose(1, 2, 0)
    return np.ascontiguousarray(W6.reshape(128, W_COLS))


def _build_xpb(x):
    """XPb[b, dd*32+i, u] = xp[b, i, 4u+dd], xp = [zeros(2048), x, zeros]."""
    xp = np.zeros((B, C_IN, 4100), np.float32)
    xp[:, :, 2048:4096] = x
    XPb = np.empty((B, 128, XPW), np.float32)
    for dd in range(4):
        XPb[:, dd * 32:(dd + 1) * 32, :] = xp[:, :, dd::4][:, :, :XPW]
    return XPb


def _w_chunks():
    """W DMA chunk boundaries in blocks: small leading chunks so the first
    matmul starts early, then steady 8-block chunks."""
    sizes = [2, 2, 4]
    while sum(sizes) < NBLK:
        sizes.append(min(W_CHUNK_BLKS, NBLK - sum(sizes)))
    bounds, acc = [], 0
    for s in sizes:
        bounds.append((acc, acc + s))
        acc += s
    return bounds


# ------------------------------------------------------- tile drain patch

def _patch_tile_drain():
    """This walrus build rejects >2 sync waits on a CTRL (Drain) instruction;
    spread the TileContext exit waits over single-wait NOPs instead."""
    from concourse.tile import TileContext
    from concourse.vector_clock import ScopedClock, VectorClock

    if getattr(TileContext, "_ck_drain_patched", False):
        return

    def _drain_and_barrier(self, tick_clock, wait_clock):
        gc = tick_clock.global_clock
        n = len(gc)
        for p in range(n):
            if gc[p] <= 0:
                continue
            vec = [gc[q] if q == p else 0 for q in range(n)]
            nop = self.nc.sync.nop(nofuse=True, hint=f"split_drain_wait_p{p}")
            wait_clock.add_sem_waits(nop.ins, ScopedClock({None: VectorClock(vec)}))
        self.nc.sync.drain()
        self.nc.all_engine_barrier()
        assert self.sems is not None
        popped = self.nc._tile_sem_poison_stack.pop()
        assert popped is self._sem_poison
        self.nc.clear_and_free_semaphores(list(self.sems.allocated().values()))
        self.nc.all_engine_barrier()

    TileContext._drain_and_barrier = _drain_and_barrier
    TileContext._ck_drain_patched = True


WAIT_LIMIT = 1  # this walrus build encodes at most 2 sync waits per instruction


def _split_excess_waits(nc, limit=WAIT_LIMIT):
    """Hoist excess sem waits onto same-engine NOPs placed just before the
    instruction - in-order engine queues make this semantically identical."""
    import concourse.mybir as mybir

    n_split = 0
    for f in nc.m.functions:
        for bb in f.blocks:
            new_insts = []
            changed = False
            for inst in bb.instructions:
                si = inst.sync_info
                waits = list(si.on_wait) if si is not None and si.on_wait else []
                if len(waits) > limit:
                    extra, keep = waits[:-limit], waits[-limit:]
                    for i in range(0, len(extra), limit):
                        n_split += 1
                        new_insts.append(
                            mybir.InstNoOp(
                                name=f"I-ckwsplit-{n_split}",
                                engine=inst.engine,
                                ins=[],
                                outs=[],
                                sync_info=mybir.SyncInfo(
                                    on_wait=extra[i : i + limit], on_update=[]
                                ),
                            )
                        )
                    inst.sync_info = mybir.SyncInfo(
                        on_wait=keep, on_update=list(si.on_update) if si.on_update else []
                    )
                    changed = True
                new_insts.append(inst)
            if changed:
                bb.instructions = new_insts
    return n_split


# ------------------------------------------------------------ device kernel

def _build_nc():
    import concourse.bass as bass
    import concourse.mybir as mybir
    from concourse.tile import TileContext

    _patch_tile_drain()
    f32 = mybir.dt.float32
    bf16 = mybir.dt.bfloat16

    nc = bass.Bass()
    xp_d = nc.declare_dram_parameter("xp", [128, XPW], bf16, isOutput=False)
    w_d = nc.declare_dram_parameter("w", [128, W_COLS], bf16, isOutput=False)
    bias_d = nc.declare_dram_parameter("bias", [128, 1], f32, isOutput=False)
    out_d = nc.declare_dram_parameter("out", [128, 512], f32, isOutput=True)

    with TileContext(nc) as tc:
        with (
            tc.tile_pool(name="const", bufs=1) as const,
            tc.tile_pool(name="work", bufs=1) as work,
            tc.tile_pool(name="acc_psum", bufs=1, space="PSUM") as acc_psum,
        ):
            xp_sb = const.tile([128, XPW], bf16)
            nc.sync.dma_start(xp_sb[:, :], xp_d[:, :])
            w_sb = const.tile([128, W_COLS], bf16)
            bounds = _w_chunks()
            lo, hi = bounds[0]
            nc.sync.dma_start(w_sb[:, lo * 128 : hi * 128], w_d[:, lo * 128 : hi * 128])
            bias_sb = const.tile([128, 1], f32)
            nc.sync.dma_start(bias_sb[:, :], bias_d[:, :])
            for lo, hi in bounds[1:]:
                nc.sync.dma_start(
                    w_sb[:, lo * 128 : hi * 128], w_d[:, lo * 128 : hi * 128]
                )

            accs = [
                acc_psum.tile([128, GW], f32, name=f"acc{g}") for g in range(NGROUPS)
            ]
            ots = [work.tile([128, GW], f32, name=f"ot{g}") for g in range(NGROUPS)]
            for bi in range(NBLK):
                m = NBLK - bi
                c_lo = 512 - m           # block m only reaches output cols >= c_lo
                for g in range(c_lo // GW, NGROUPS):
                    lo = max(c_lo, g * GW)
                    hi = (g + 1) * GW
                    last_for_g = m == max(1, 512 - (hi - 1))
                    nc.tensor.matmul(
                        accs[g][:, lo - g * GW : hi - g * GW],
                        w_sb[:, bi * 128 : (bi + 1) * 128],
                        xp_sb[:, lo + m : hi + m],
                        start=(m == NBLK),
                        stop=last_for_g,
                        skip_group_check=True,
                    )
                    if last_for_g:
                        nc.vector.tensor_scalar_add(
                            ots[g][:, :], accs[g][:, :], bias_sb[:, :]
                        )
                        nc.sync.dma_start(
                            out_d[:, g * GW : (g + 1) * GW], ots[g][:, :]
                        )
    _split_excess_waits(nc)
    return nc


# ------------------------------------------------------------------- entry

def kernel(**inputs):
    from concourse.bass_utils import run_bass_kernel_spmd
    import ml_dtypes

    x = np.asarray(inputs["x"], dtype=np.float32)
    K = _siren_kernel(
        np.asarray(inputs["pos_rel"]), np.asarray(inputs["w1"]),
        np.asarray(inputs["b1"]), np.asarray(inputs["w2"]),
        np.asarray(inputs["b2"]), np.asarray(inputs["w3"]),
        np.asarray(inputs["b3"]),
    )
    W = _build_w(K).astype(ml_dtypes.bfloat16)
    XPb = _build_xpb(x).astype(ml_dtypes.bfloat16)
    bias128 = np.ascontiguousarray(
        np.tile(np.asarray(inputs["bias"], dtype=np.float32), 4).reshape(128, 1)
    )

    if "nc" not in _cache:
        _cache["nc"] = _build_nc()
    nc = _cache["nc"]

    n_cores = int(os.environ.get("CK_CORES", str(N_CORES)))
    in_maps = [
        {"xp": XPb[b % B], "w": W, "bias": bias128} for b in range(n_cores)
    ]

    def unpack(arr):
        return arr.reshape(4, 32, 512).transpose(1, 2, 0).reshape(32, T)

    # The axon-tunneled device occasionally throws a transient
    # NRT_EXEC_UNIT_UNRECOVERABLE on 8-core launches; retry, then fall back
    # to two 4-core waves (same NEFF, batches split across waves).
    res = None
    for attempt in range(3):
        try:
            res = run_bass_kernel_spmd(nc, in_maps, core_ids=list(range(n_cores)))
            break
        except Exception:
            if attempt == 2:
                res = None
            else:
                continue
    if res is not None:
        out = np.stack(
            [unpack(res.results[b % n_cores]["out"]) for b in range(B)], axis=0
        )
        return out.astype(np.float32)

    half = n_cores // 2 if n_cores > 1 else 1
    outs = []
    for w0 in range(0, B, half):
        wave_maps = [
            {"xp": XPb[(w0 + c) % B], "w": W, "bias": bias128}
            for c in range(half)
        ]
        wres = run_bass_kernel_spmd(nc, wave_maps, core_ids=list(range(half)))
        outs.extend(unpack(wres.results[c]["out"]) for c in range(half))
    out = np.stack(outs[:B], axis=0)
    return out.astype(np.float32)


# revision 8
# speedup vs baseline: 4.8439x; 1.0442x over previous
"""CKConv (nn_CKConv_85950885527678) Trainium2 Bass kernel.

Strategy: data-parallel over batch (8 batches -> 8 NeuronCores). The tiny
SIREN kernel network is evaluated on the host; the generated conv kernel is
replicated to every core (per the sharding hint).

Per core the causal conv out[o,t] = sum_{i,l} K[o,i,l] * xp[i,t+l]
(xp = x left-padded with 2048 zeros; taps l in [1,2048]) is computed with
full-width M=128 matmuls via time-decimation by 4:

  - output element (r,o) at psum column c holds out[o, 4c+r]
    (r in 0..3 output-time residue, o in 0..31 out channel)
  - moving operand XPb[(dd,i), v] = x[i, 4v+dd] - a stride-4 view of x whose
    column shift by m advances all taps by 4m (the zero-pad region is never
    read, so XPb is just a reshape of x)
  - stationary block m (m = 512..1): W_m[(dd,i),(r,o)] = K[o,i,4m+dd-r]
    (zero out of range); psum accumulates over m
  - causality: block m contributes only to output cols c >= 512-m, so each
    block issues variable-width matmuls covering exactly [512-m, 512),
    split at psum-group boundaries (4 psum banks of 128 cols, drained +
    bias-added as they complete)

Every matmul is M=128 x K=128, so the PE array is fully used and the charged
moving columns hit the causal lower bound (~131k col-cycles). The weight
blocks are streamed (16.8 MB bf16) in block order matching first use so the
DMA hides under the matmul stream; the first two blocks ride in the same DMA
as XPb so the PE starts as early as possible.
"""

import os
import numpy as np

B, C_IN, C_OUT, T, D = 8, 32, 32, 2048, 32
L = T + 1
NBLK = 512          # tap blocks m = 512..1 (block index b = 512-m)
GB = (0, 128, 256, 384, 512)   # psum group col boundaries (4 banks)
XPW = 512           # XPb columns (v = c+m-512 in [0, 512))
FUSE = 2            # W blocks fused into the first (xp) DMA
W_COLS = NBLK * 128
LEAD = (4, 6)       # first standalone W chunk sizes (blocks), then 8s
BIAS_AFTER = 4      # bias DMA issued after this many standalone W chunks
N_CORES = 8

_cache = {}


# ---------------------------------------------------------------- host prep

def _siren_kernel(pos_rel, w1, b1, w2, b2, w3, b3):
    p = pos_rel.reshape(1, L).astype(np.float32)
    h = np.sin(w1.astype(np.float32) @ p + b1[:, None].astype(np.float32))
    h = np.sin(w2.astype(np.float32) @ h + b2[:, None].astype(np.float32))
    k = w3.astype(np.float32) @ h + b3[:, None].astype(np.float32)
    return k.reshape(C_OUT, C_IN, L).astype(np.float32)


def _build_w(K):
    """W[dd*32+i, b*128 + r*32+o] = K[o,i,4m+dd-r], m = 512-b."""
    ms = np.arange(NBLK, 0, -1)
    W6 = np.zeros((4, C_IN, NBLK, 4, C_OUT), np.float32)
    for dd in range(4):
        for r in range(4):
            l = 4 * ms + dd - r
            valid = (l >= 1) & (l <= 2048)
            src = K[:, :, np.clip(l, 0, 2048)]          # (o, i, b)
            src = np.where(valid[None, None, :], src, 0.0)
            W6[dd, :, :, r, :] = src.transpose(1, 2, 0)
    return np.ascontiguousarray(W6.reshape(128, W_COLS))


def _build_xpb(x):
    """XPb[b, dd*32+i, v] = x[b, i, 4v+dd] - stride-4 reshape, no padding."""
    XPb = np.empty((B, 128, XPW), np.float32)
    for dd in range(4):
        XPb[:, dd * 32:(dd + 1) * 32, :] = x[:, :, dd::4]
    return XPb


def _w_chunks():
    """Standalone W DMA chunk boundaries in blocks (after the FUSE blocks)."""
    sizes = list(LEAD)
    rem = NBLK - FUSE
    while sum(sizes) < rem:
        sizes.append(min(8, rem - sum(sizes)))
    bounds, acc = [], 0
    for s in sizes:
        bounds.append((acc, acc + s))
        acc += s
    return bounds


# ------------------------------------------------------- tile drain patch

def _patch_tile_drain():
    """This walrus build rejects >2 sync waits on a CTRL (Drain) instruction;
    spread the TileContext exit waits over single-wait NOPs instead."""
    from concourse.tile import TileContext
    from concourse.vector_clock import ScopedClock, VectorClock

    if getattr(TileContext, "_ck_drain_patched", False):
        return

    def _drain_and_barrier(self, tick_clock, wait_clock):
        gc = tick_clock.global_clock
        n = len(gc)
        for p in range(n):
            if gc[p] <= 0:
                continue
            vec = [gc[q] if q == p else 0 for q in range(n)]
            nop = self.nc.sync.nop(nofuse=True, hint=f"split_drain_wait_p{p}")
            wait_clock.add_sem_waits(nop.ins, ScopedClock({None: VectorClock(vec)}))
        self.nc.sync.drain()
        self.nc.all_engine_barrier()
        assert self.sems is not None
        popped = self.nc._tile_sem_poison_stack.pop()
        assert popped is self._sem_poison
        self.nc.clear_and_free_semaphores(list(self.sems.allocated().values()))
        self.nc.all_engine_barrier()

    TileContext._drain_and_barrier = _drain_and_barrier
    TileContext._ck_drain_patched = True


WAIT_LIMIT = 1  # this walrus build encodes at most 2 sync waits per instruction


def _split_excess_waits(nc, limit=WAIT_LIMIT):
    """Hoist excess sem waits onto same-engine NOPs placed just before the
    instruction - in-order engine queues make this semantically identical."""
    import concourse.mybir as mybir

    n_split = 0
    for f in nc.m.functions:
        for bb in f.blocks:
            new_insts = []
            changed = False
            for inst in bb.instructions:
                si = inst.sync_info
                waits = list(si.on_wait) if si is not None and si.on_wait else []
                if len(waits) > limit:
                    extra, keep = waits[:-limit], waits[-limit:]
                    for i in range(0, len(extra), limit):
                        n_split += 1
                        new_insts.append(
                            mybir.InstNoOp(
                                name=f"I-ckwsplit-{n_split}",
                                engine=inst.engine,
                                ins=[],
                                outs=[],
                                sync_info=mybir.SyncInfo(
                                    on_wait=extra[i : i + limit], on_update=[]
                                ),
                            )
                        )
                    inst.sync_info = mybir.SyncInfo(
                        on_wait=keep, on_update=list(si.on_update) if si.on_update else []
                    )
                    changed = True
                new_insts.append(inst)
            if changed:
                bb.instructions = new_insts
    return n_split


# ------------------------------------------------------------ device kernel

def _build_nc():
    import concourse.bass as bass
    import concourse.mybir as mybir
    from concourse.tile import TileContext

    _patch_tile_drain()
    f32 = mybir.dt.float32
    bf16 = mybir.dt.bfloat16

    nc = bass.Bass()
    xw_d = nc.declare_dram_parameter(
        "xw0", [128, XPW + FUSE * 128], bf16, isOutput=False
    )
    w_d = nc.declare_dram_parameter(
        "w", [128, W_COLS - FUSE * 128], bf16, isOutput=False
    )
    bias_d = nc.declare_dram_parameter("bias", [128, 1], f32, isOutput=False)
    out_d = nc.declare_dram_parameter("out", [128, 512], f32, isOutput=True)

    with TileContext(nc) as tc:
        with (
            tc.tile_pool(name="const", bufs=1) as const,
            tc.tile_pool(name="work", bufs=1) as work,
            tc.tile_pool(name="acc_psum", bufs=1, space="PSUM") as acc_psum,
        ):
            xw_sb = const.tile([128, XPW + FUSE * 128], bf16)
            nc.sync.dma_start(xw_sb[:, :], xw_d[:, :])
            w_sb = const.tile([128, W_COLS - FUSE * 128], bf16)
            bias_sb = const.tile([128, 1], f32)
            for k, (lo, hi) in enumerate(_w_chunks()):
                nc.sync.dma_start(
                    w_sb[:, lo * 128 : hi * 128], w_d[:, lo * 128 : hi * 128]
                )
                if k == BIAS_AFTER:
                    nc.sync.dma_start(bias_sb[:, :], bias_d[:, :])

            accs = [
                acc_psum.tile([128, 128], f32, name=f"acc{g}") for g in range(4)
            ]
            ots = [work.tile([128, 128], f32, name=f"ot{g}") for g in range(4)]

            def w_slice(bi):
                if bi < FUSE:
                    return xw_sb[:, XPW + bi * 128 : XPW + (bi + 1) * 128]
                b = bi - FUSE
                return w_sb[:, b * 128 : (b + 1) * 128]

            for bi in range(NBLK):
                m = NBLK - bi
                c_lo = 512 - m           # block m only reaches output cols >= c_lo
                for g in range(4):
                    glo, ghi = GB[g], GB[g + 1]
                    if ghi <= c_lo:
                        continue
                    lo = max(c_lo, glo)
                    last_for_g = m == max(1, 512 - (ghi - 1))
                    nc.tensor.matmul(
                        accs[g][:, lo - glo : ghi - glo],
                        w_slice(bi),
                        xw_sb[:, lo + m - 512 : ghi + m - 512],
                        start=(m == NBLK),
                        stop=last_for_g,
                        skip_group_check=True,
                    )
                    if last_for_g:
                        nc.vector.tensor_scalar_add(
                            ots[g][:, :], accs[g][:, :], bias_sb[:, :]
                        )
                        nc.sync.dma_start(
                            out_d[:, glo : ghi], ots[g][:, :]
                        )
    _split_excess_waits(nc)
    return nc


# ------------------------------------------------------------------- entry

def kernel(**inputs):
    from concourse.bass_utils import run_bass_kernel_spmd
    import ml_dtypes

    x = np.asarray(inputs["x"], dtype=np.float32)
    K = _siren_kernel(
        np.asarray(inputs["pos_rel"]), np.asarray(inputs["w1"]),
        np.asarray(inputs["b1"]), np.asarray(inputs["w2"]),
        np.asarray(inputs["b2"]), np.asarray(inputs["w3"]),
        np.asarray(inputs["b3"]),
    )
    W = _build_w(K).astype(ml_dtypes.bfloat16)
    XPb = _build_xpb(x).astype(ml_dtypes.bfloat16)
    Wpre, Wrest = W[:, : FUSE * 128], np.ascontiguousarray(W[:, FUSE * 128 :])
    bias128 = np.ascontiguousarray(
        np.tile(np.asarray(inputs["bias"], dtype=np.float32), 4).reshape(128, 1)
    )

    if "nc" not in _cache:
        _cache["nc"] = _build_nc()
    nc = _cache["nc"]

    n_cores = int(os.environ.get("CK_CORES", str(N_CORES)))
    xw = [
        np.ascontiguousarray(np.concatenate([XPb[b % B], Wpre], axis=1))
        for b in range(n_cores)
    ]
    in_maps = [
        {"xw0": xw[b], "w": Wrest, "bias": bias128} for b in range(n_cores)
    ]

    def unpack(arr):
        return arr.reshape(4, 32, 512).transpose(1, 2, 0).reshape(32, T)

    # The axon-tunneled device occasionally throws a transient
    # NRT_EXEC_UNIT_UNRECOVERABLE on 8-core launches; retry, then fall back
    # to two 4-core waves (same NEFF, batches split across waves).
    res = None
    for attempt in range(3):
        try:
            res = run_bass_kernel_spmd(nc, in_maps, core_ids=list(range(n_cores)))
            break
        except Exception:
            if attempt == 2:
                res = None
            else:
                continue
    if res is not None:
        out = np.stack(
            [unpack(res.results[b % n_cores]["out"]) for b in range(B)], axis=0
        )
        return out.astype(np.float32)

    half = n_cores // 2 if n_cores > 1 else 1
    outs = []
    for w0 in range(0, B, half):
        wave_maps = [
            {"xw0": xw[(w0 + c) % len(xw)], "w": Wrest, "bias": bias128}
            for c in range(half)
        ]
        wres = run_bass_kernel_spmd(nc, wave_maps, core_ids=list(range(half)))
        outs.extend(unpack(wres.results[c]["out"]) for c in range(half))
    out = np.stack(outs[:B], axis=0)
    return out.astype(np.float32)


# revision 10
# speedup vs baseline: 4.8479x; 1.0008x over previous
"""CKConv (nn_CKConv_85950885527678) Trainium2 Bass kernel.

Strategy: data-parallel over batch (8 batches -> 8 NeuronCores). The tiny
SIREN kernel network is evaluated on the host; the generated conv kernel is
replicated to every core (per the sharding hint).

Per core the causal conv out[o,t] = sum_{i,l} K[o,i,l] * xp[i,t+l]
(xp = x left-padded with 2048 zeros; taps l in [1,2048]) is computed with
full-width M=128 matmuls via time-decimation by 4:

  - output element (r,o) at psum column c holds out[o, 4c+r]
    (r in 0..3 output-time residue, o in 0..31 out channel)
  - moving operand XPb[(dd,i), v] = x[i, 4v+dd] - a stride-4 view of x whose
    column shift by m advances all taps by 4m (the zero-pad region is never
    read, so XPb is just a reshape of x)
  - stationary block m (m = 512..1): W_m[(dd,i),(r,o)] = K[o,i,4m+dd-r]
    (zero out of range); psum accumulates over m
  - causality: block m contributes only to output cols c >= 512-m, so each
    block issues variable-width matmuls covering exactly [512-m, 512),
    split at psum-group boundaries (4 psum banks of 128 cols, drained +
    bias-added as they complete)

Every matmul is M=128 x K=128, so the PE array is fully used and the charged
moving columns hit the causal lower bound (~131k col-cycles). The weight
blocks are streamed (16.8 MB bf16) in block order matching first use so the
DMA hides under the matmul stream; the first two blocks ride in the same DMA
as XPb so the PE starts as early as possible.
"""

import os
import numpy as np

B, C_IN, C_OUT, T, D = 8, 32, 32, 2048, 32
L = T + 1
NBLK = 512          # tap blocks m = 512..1 (block index b = 512-m)
GB = (0, 128, 256, 384, 512)   # psum group col boundaries (4 banks)
XPW = 512           # XPb columns (v = c+m-512 in [0, 512))
FUSE = 2            # W blocks fused into the first (xp) DMA
W_COLS = NBLK * 128
LEAD = (4, 6)       # first standalone W chunk sizes (blocks), then STEADY
STEADY = 9          # steady-state W DMA chunk size (blocks)
BIAS_AFTER = 4      # bias DMA issued after this many standalone W chunks
N_CORES = 8

_cache = {}


# ---------------------------------------------------------------- host prep

def _siren_kernel(pos_rel, w1, b1, w2, b2, w3, b3):
    p = pos_rel.reshape(1, L).astype(np.float32)
    h = np.sin(w1.astype(np.float32) @ p + b1[:, None].astype(np.float32))
    h = np.sin(w2.astype(np.float32) @ h + b2[:, None].astype(np.float32))
    k = w3.astype(np.float32) @ h + b3[:, None].astype(np.float32)
    return k.reshape(C_OUT, C_IN, L).astype(np.float32)


def _build_w(K):
    """W[dd*32+i, b*128 + r*32+o] = K[o,i,4m+dd-r], m = 512-b."""
    ms = np.arange(NBLK, 0, -1)
    W6 = np.zeros((4, C_IN, NBLK, 4, C_OUT), np.float32)
    for dd in range(4):
        for r in range(4):
            l = 4 * ms + dd - r
            valid = (l >= 1) & (l <= 2048)
            src = K[:, :, np.clip(l, 0, 2048)]          # (o, i, b)
            src = np.where(valid[None, None, :], src, 0.0)
            W6[dd, :, :, r, :] = src.transpose(1, 2, 0)
    return np.ascontiguousarray(W6.reshape(128, W_COLS))


def _build_xpb(x):
    """XPb[b, dd*32+i, v] = x[b, i, 4v+dd] - stride-4 reshape, no padding."""
    XPb = np.empty((B, 128, XPW), np.float32)
    for dd in range(4):
        XPb[:, dd * 32:(dd + 1) * 32, :] = x[:, :, dd::4]
    return XPb


def _w_chunks():
    """Standalone W DMA chunk boundaries in blocks (after the FUSE blocks)."""
    sizes = list(LEAD)
    rem = NBLK - FUSE
    while sum(sizes) < rem:
        sizes.append(min(STEADY, rem - sum(sizes)))
    bounds, acc = [], 0
    for s in sizes:
        bounds.append((acc, acc + s))
        acc += s
    return bounds


# ------------------------------------------------------- tile drain patch

def _patch_tile_drain():
    """This walrus build rejects >2 sync waits on a CTRL (Drain) instruction;
    spread the TileContext exit waits over single-wait NOPs instead."""
    from concourse.tile import TileContext
    from concourse.vector_clock import ScopedClock, VectorClock

    if getattr(TileContext, "_ck_drain_patched", False):
        return

    def _drain_and_barrier(self, tick_clock, wait_clock):
        gc = tick_clock.global_clock
        n = len(gc)
        for p in range(n):
            if gc[p] <= 0:
                continue
            vec = [gc[q] if q == p else 0 for q in range(n)]
            nop = self.nc.sync.nop(nofuse=True, hint=f"split_drain_wait_p{p}")
            wait_clock.add_sem_waits(nop.ins, ScopedClock({None: VectorClock(vec)}))
        self.nc.sync.drain()
        self.nc.all_engine_barrier()
        assert self.sems is not None
        popped = self.nc._tile_sem_poison_stack.pop()
        assert popped is self._sem_poison
        self.nc.clear_and_free_semaphores(list(self.sems.allocated().values()))
        self.nc.all_engine_barrier()

    TileContext._drain_and_barrier = _drain_and_barrier
    TileContext._ck_drain_patched = True


WAIT_LIMIT = 1  # this walrus build encodes at most 2 sync waits per instruction


def _split_excess_waits(nc, limit=WAIT_LIMIT):
    """Hoist excess sem waits onto same-engine NOPs placed just before the
    instruction - in-order engine queues make this semantically identical."""
    import concourse.mybir as mybir

    n_split = 0
    for f in nc.m.functions:
        for bb in f.blocks:
            new_insts = []
            changed = False
            for inst in bb.instructions:
                si = inst.sync_info
                waits = list(si.on_wait) if si is not None and si.on_wait else []
                if len(waits) > limit:
                    extra, keep = waits[:-limit], waits[-limit:]
                    for i in range(0, len(extra), limit):
                        n_split += 1
                        new_insts.append(
                            mybir.InstNoOp(
                                name=f"I-ckwsplit-{n_split}",
                                engine=inst.engine,
                                ins=[],
                                outs=[],
                                sync_info=mybir.SyncInfo(
                                    on_wait=extra[i : i + limit], on_update=[]
                                ),
                            )
                        )
                    inst.sync_info = mybir.SyncInfo(
                        on_wait=keep, on_update=list(si.on_update) if si.on_update else []
                    )
                    changed = True
                new_insts.append(inst)
            if changed:
                bb.instructions = new_insts
    return n_split


# ------------------------------------------------------------ device kernel

def _build_nc():
    import concourse.bass as bass
    import concourse.mybir as mybir
    from concourse.tile import TileContext

    _patch_tile_drain()
    f32 = mybir.dt.float32
    bf16 = mybir.dt.bfloat16

    nc = bass.Bass()
    xw_d = nc.declare_dram_parameter(
        "xw0", [128, XPW + FUSE * 128], bf16, isOutput=False
    )
    w_d = nc.declare_dram_parameter(
        "w", [128, W_COLS - FUSE * 128], bf16, isOutput=False
    )
    bias_d = nc.declare_dram_parameter("bias", [128, 1], f32, isOutput=False)
    out_d = nc.declare_dram_parameter("out", [128, 512], f32, isOutput=True)

    with TileContext(nc) as tc:
        with (
            tc.tile_pool(name="const", bufs=1) as const,
            tc.tile_pool(name="work", bufs=1) as work,
            tc.tile_pool(name="acc_psum", bufs=1, space="PSUM") as acc_psum,
        ):
            xw_sb = const.tile([128, XPW + FUSE * 128], bf16)
            nc.sync.dma_start(xw_sb[:, :], xw_d[:, :])
            w_sb = const.tile([128, W_COLS - FUSE * 128], bf16)
            bias_sb = const.tile([128, 1], f32)
            for k, (lo, hi) in enumerate(_w_chunks()):
                nc.sync.dma_start(
                    w_sb[:, lo * 128 : hi * 128], w_d[:, lo * 128 : hi * 128]
                )
                if k == BIAS_AFTER:
                    nc.sync.dma_start(bias_sb[:, :], bias_d[:, :])

            accs = [
                acc_psum.tile([128, 128], f32, name=f"acc{g}") for g in range(4)
            ]
            ots = [work.tile([128, 128], f32, name=f"ot{g}") for g in range(4)]

            def w_slice(bi):
                if bi < FUSE:
                    return xw_sb[:, XPW + bi * 128 : XPW + (bi + 1) * 128]
                b = bi - FUSE
                return w_sb[:, b * 128 : (b + 1) * 128]

            for bi in range(NBLK):
                m = NBLK - bi
                c_lo = 512 - m           # block m only reaches output cols >= c_lo
                for g in range(4):
                    glo, ghi = GB[g], GB[g + 1]
                    if ghi <= c_lo:
                        continue
                    lo = max(c_lo, glo)
                    last_for_g = m == max(1, 512 - (ghi - 1))
                    nc.tensor.matmul(
                        accs[g][:, lo - glo : ghi - glo],
                        w_slice(bi),
                        xw_sb[:, lo + m - 512 : ghi + m - 512],
                        start=(m == NBLK),
                        stop=last_for_g,
                        skip_group_check=True,
                    )
                    if last_for_g:
                        nc.vector.tensor_scalar_add(
                            ots[g][:, :], accs[g][:, :], bias_sb[:, :]
                        )
                        nc.sync.dma_start(
                            out_d[:, glo : ghi], ots[g][:, :]
                        )
    _split_excess_waits(nc)
    return nc


# ------------------------------------------------------------------- entry

def kernel(**inputs):
    from concourse.bass_utils import run_bass_kernel_spmd
    import ml_dtypes

    x = np.asarray(inputs["x"], dtype=np.float32)
    K = _siren_kernel(
        np.asarray(inputs["pos_rel"]), np.asarray(inputs["w1"]),
        np.asarray(inputs["b1"]), np.asarray(inputs["w2"]),
        np.asarray(inputs["b2"]), np.asarray(inputs["w3"]),
        np.asarray(inputs["b3"]),
    )
    W = _build_w(K).astype(ml_dtypes.bfloat16)
    XPb = _build_xpb(x).astype(ml_dtypes.bfloat16)
    Wpre, Wrest = W[:, : FUSE * 128], np.ascontiguousarray(W[:, FUSE * 128 :])
    bias128 = np.ascontiguousarray(
        np.tile(np.asarray(inputs["bias"], dtype=np.float32), 4).reshape(128, 1)
    )

    if "nc" not in _cache:
        _cache["nc"] = _build_nc()
    nc = _cache["nc"]

    n_cores = int(os.environ.get("CK_CORES", str(N_CORES)))
    xw = [
        np.ascontiguousarray(np.concatenate([XPb[b % B], Wpre], axis=1))
        for b in range(n_cores)
    ]
    in_maps = [
        {"xw0": xw[b], "w": Wrest, "bias": bias128} for b in range(n_cores)
    ]

    def unpack(arr):
        return arr.reshape(4, 32, 512).transpose(1, 2, 0).reshape(32, T)

    # The axon-tunneled device occasionally throws a transient
    # NRT_EXEC_UNIT_UNRECOVERABLE on 8-core launches; retry, then fall back
    # to two 4-core waves (same NEFF, batches split across waves).
    res = None
    for attempt in range(3):
        try:
            res = run_bass_kernel_spmd(nc, in_maps, core_ids=list(range(n_cores)))
            break
        except Exception:
            if attempt == 2:
                res = None
            else:
                continue
    if res is not None:
        out = np.stack(
            [unpack(res.results[b % n_cores]["out"]) for b in range(B)], axis=0
        )
        return out.astype(np.float32)

    half = n_cores // 2 if n_cores > 1 else 1
    outs = []
    for w0 in range(0, B, half):
        wave_maps = [
            {"xw0": xw[(w0 + c) % len(xw)], "w": Wrest, "bias": bias128}
            for c in range(half)
        ]
        wres = run_bass_kernel_spmd(nc, wave_maps, core_ids=list(range(half)))
        outs.extend(unpack(wres.results[c]["out"]) for c in range(half))
    out = np.stack(outs[:B], axis=0)
    return out.astype(np.float32)


# revision 13
# speedup vs baseline: 4.8884x; 1.0084x over previous
"""CKConv (nn_CKConv_85950885527678) Trainium2 Bass kernel.

Strategy: data-parallel over batch (8 batches -> 8 NeuronCores). The tiny
SIREN kernel network is evaluated on the host; the generated conv kernel is
replicated to every core (per the sharding hint).

Per core the causal conv out[o,t] = sum_{i,l} K[o,i,l] * xp[i,t+l]
(xp = x left-padded with 2048 zeros; taps l in [1,2048]) is computed with
full-width M=128 matmuls via time-decimation by 4:

  - output element (r,o) at psum column c holds out[o, 4c+r]
    (r in 0..3 output-time residue, o in 0..31 out channel)
  - moving operand XPb[(dd,i), v] = x[i, 4v+dd] - a stride-4 view of x whose
    column shift by m advances all taps by 4m (the zero-pad region is never
    read, so XPb is just a reshape of x)
  - stationary block m (m = 512..1): W_m[(dd,i),(r,o)] = K[o,i,4m+dd-r]
    (zero out of range); psum accumulates over m
  - causality: block m contributes only to output cols c >= 512-m, so each
    block issues variable-width matmuls covering exactly [512-m, 512),
    split at psum-group boundaries (4 psum banks of 128 cols, drained +
    bias-added as they complete)

Every matmul is M=128 x K=128, so the PE array is fully used and the charged
moving columns hit the causal lower bound (~131k col-cycles). The weight
blocks are streamed (16.8 MB bf16) in block order matching first use so the
DMA hides under the matmul stream; the first two blocks ride in the same DMA
as XPb so the PE starts as early as possible.
"""

import os
import numpy as np

B, C_IN, C_OUT, T, D = 8, 32, 32, 2048, 32
L = T + 1
NBLK = 512          # tap blocks m = 512..1 (block index b = 512-m)
GB = (0, 128, 256, 384, 512)   # psum group col boundaries (4 banks)
XPW = 512           # XPb columns (v = c+m-512 in [0, 512))
FUSE = 2            # W blocks fused into the first (xp) DMA
W_COLS = NBLK * 128
LEAD = (4, 6)       # first standalone W chunk sizes (blocks), then STEADY
STEADY = 9          # steady-state W DMA chunk size (blocks)
BIAS_AFTER = 4      # bias DMA issued after this many standalone W chunks
N_CORES = 8

_cache = {}


# ---------------------------------------------------------------- host prep

def _siren_kernel(pos_rel, w1, b1, w2, b2, w3, b3):
    p = pos_rel.reshape(1, L).astype(np.float32)
    h = np.sin(w1.astype(np.float32) @ p + b1[:, None].astype(np.float32))
    h = np.sin(w2.astype(np.float32) @ h + b2[:, None].astype(np.float32))
    k = w3.astype(np.float32) @ h + b3[:, None].astype(np.float32)
    return k.reshape(C_OUT, C_IN, L).astype(np.float32)


def _build_w(K):
    """W[dd*32+i, b*128 + r*32+o] = K[o,i,4m+dd-r], m = 512-b."""
    ms = np.arange(NBLK, 0, -1)
    W6 = np.zeros((4, C_IN, NBLK, 4, C_OUT), np.float32)
    for dd in range(4):
        for r in range(4):
            l = 4 * ms + dd - r
            valid = (l >= 1) & (l <= 2048)
            src = K[:, :, np.clip(l, 0, 2048)]          # (o, i, b)
            src = np.where(valid[None, None, :], src, 0.0)
            W6[dd, :, :, r, :] = src.transpose(1, 2, 0)
    return np.ascontiguousarray(W6.reshape(128, W_COLS))


def _build_xpb(x):
    """XPb[b, dd*32+i, v] = x[b, i, 4v+dd] - stride-4 reshape, no padding."""
    XPb = np.empty((B, 128, XPW), np.float32)
    for dd in range(4):
        XPb[:, dd * 32:(dd + 1) * 32, :] = x[:, :, dd::4]
    return XPb


def _w_chunks():
    """Standalone W DMA chunk boundaries in blocks (after the FUSE blocks)."""
    sizes = list(LEAD)
    rem = NBLK - FUSE
    while sum(sizes) < rem:
        sizes.append(min(STEADY, rem - sum(sizes)))
    bounds, acc = [], 0
    for s in sizes:
        bounds.append((acc, acc + s))
        acc += s
    return bounds


# ------------------------------------------------------- tile drain patch

def _patch_tile_drain():
    """This walrus build rejects >2 sync waits on a CTRL (Drain) instruction;
    spread the TileContext exit waits over single-wait NOPs instead. Also
    drop the exit-time semaphore clear + second barrier: every kernel() call
    executes a freshly-loaded NEFF (sems start zeroed), and nothing runs
    after this single TileContext, so the reset work only adds tail latency."""
    from concourse.tile import TileContext
    from concourse.vector_clock import ScopedClock, VectorClock

    if getattr(TileContext, "_ck_drain_patched", False):
        return

    def _drain_and_barrier(self, tick_clock, wait_clock):
        gc = tick_clock.global_clock
        n = len(gc)
        for p in range(n):
            if gc[p] <= 0:
                continue
            vec = [gc[q] if q == p else 0 for q in range(n)]
            nop = self.nc.sync.nop(nofuse=True, hint=f"split_drain_wait_p{p}")
            wait_clock.add_sem_waits(nop.ins, ScopedClock({None: VectorClock(vec)}))
        self.nc.sync.drain()
        self.nc.all_engine_barrier()
        assert self.sems is not None
        popped = self.nc._tile_sem_poison_stack.pop()
        assert popped is self._sem_poison

    TileContext._drain_and_barrier = _drain_and_barrier
    TileContext._ck_drain_patched = True


WAIT_LIMIT = 1  # this walrus build encodes at most 2 sync waits per instruction


def _split_excess_waits(nc, limit=WAIT_LIMIT):
    """Hoist excess sem waits onto same-engine NOPs placed just before the
    instruction - in-order engine queues make this semantically identical."""
    import concourse.mybir as mybir

    n_split = 0
    for f in nc.m.functions:
        for bb in f.blocks:
            new_insts = []
            changed = False
            for inst in bb.instructions:
                si = inst.sync_info
                waits = list(si.on_wait) if si is not None and si.on_wait else []
                if len(waits) > limit:
                    extra, keep = waits[:-limit], waits[-limit:]
                    for i in range(0, len(extra), limit):
                        n_split += 1
                        new_insts.append(
                            mybir.InstNoOp(
                                name=f"I-ckwsplit-{n_split}",
                                engine=inst.engine,
                                ins=[],
                                outs=[],
                                sync_info=mybir.SyncInfo(
                                    on_wait=extra[i : i + limit], on_update=[]
                                ),
                            )
                        )
                    inst.sync_info = mybir.SyncInfo(
                        on_wait=keep, on_update=list(si.on_update) if si.on_update else []
                    )
                    changed = True
                new_insts.append(inst)
            if changed:
                bb.instructions = new_insts
    return n_split


def _strip_const_memsets(nc):
    """Drop the Bass preamble memsets that initialize the const-AP scratch
    (0.0 / 1.0 / 127 constants). They sit on the entry critical path (Pool
    engine gates the entry barrier) and nothing in this kernel reads them -
    const APs only back float biases of nc.scalar.activation."""
    import concourse.mybir as mybir

    n = 0
    for f in nc.m.functions:
        for bb in f.blocks:
            keep = []
            for inst in bb.instructions:
                if (
                    isinstance(inst, mybir.InstMemset)
                    and inst.sync_info is None
                    and inst.outs
                    and "const-" in str(inst.outs[0])
                ):
                    n += 1
                    continue
                keep.append(inst)
            bb.instructions = keep
    return n


# ------------------------------------------------------------ device kernel

def _build_nc():
    import concourse.bass as bass
    import concourse.mybir as mybir
    from concourse.tile import TileContext

    _patch_tile_drain()
    f32 = mybir.dt.float32
    bf16 = mybir.dt.bfloat16

    nc = bass.Bass()
    xw_d = nc.declare_dram_parameter(
        "xw0", [128, XPW + FUSE * 128], bf16, isOutput=False
    )
    w_d = nc.declare_dram_parameter(
        "w", [128, W_COLS - FUSE * 128], bf16, isOutput=False
    )
    bias_d = nc.declare_dram_parameter("bias", [128, 1], f32, isOutput=False)
    out_d = nc.declare_dram_parameter("out", [128, 512], f32, isOutput=True)

    with TileContext(nc) as tc:
        with (
            tc.tile_pool(name="const", bufs=1) as const,
            tc.tile_pool(name="work", bufs=1) as work,
            tc.tile_pool(name="acc_psum", bufs=1, space="PSUM") as acc_psum,
        ):
            xw_sb = const.tile([128, XPW + FUSE * 128], bf16)
            nc.sync.dma_start(xw_sb[:, :], xw_d[:, :])
            w_sb = const.tile([128, W_COLS - FUSE * 128], bf16)
            bias_sb = const.tile([128, 1], f32)
            for k, (lo, hi) in enumerate(_w_chunks()):
                nc.sync.dma_start(
                    w_sb[:, lo * 128 : hi * 128], w_d[:, lo * 128 : hi * 128]
                )
                if k == BIAS_AFTER:
                    nc.sync.dma_start(bias_sb[:, :], bias_d[:, :])

            accs = [
                acc_psum.tile([128, 128], f32, name=f"acc{g}") for g in range(4)
            ]
            ots = [work.tile([128, 128], f32, name=f"ot{g}") for g in range(4)]

            def w_slice(bi):
                if bi < FUSE:
                    return xw_sb[:, XPW + bi * 128 : XPW + (bi + 1) * 128]
                b = bi - FUSE
                return w_sb[:, b * 128 : (b + 1) * 128]

            for bi in range(NBLK):
                m = NBLK - bi
                c_lo = 512 - m           # block m only reaches output cols >= c_lo
                for g in range(4):
                    glo, ghi = GB[g], GB[g + 1]
                    if ghi <= c_lo:
                        continue
                    lo = max(c_lo, glo)
                    last_for_g = m == max(1, 512 - (ghi - 1))
                    nc.tensor.matmul(
                        accs[g][:, lo - glo : ghi - glo],
                        w_slice(bi),
                        xw_sb[:, lo + m - 512 : ghi + m - 512],
                        start=(m == NBLK),
                        stop=last_for_g,
                        skip_group_check=True,
                    )
                    if last_for_g:
                        nc.vector.tensor_scalar_add(
                            ots[g][:, :], accs[g][:, :], bias_sb[:, :]
                        )
                        nc.sync.dma_start(
                            out_d[:, glo : ghi], ots[g][:, :]
                        )
    _split_excess_waits(nc)
    _strip_const_memsets(nc)
    return nc


# ------------------------------------------------------------------- entry

def kernel(**inputs):
    from concourse.bass_utils import run_bass_kernel_spmd
    import ml_dtypes

    x = np.asarray(inputs["x"], dtype=np.float32)
    K = _siren_kernel(
        np.asarray(inputs["pos_rel"]), np.asarray(inputs["w1"]),
        np.asarray(inputs["b1"]), np.asarray(inputs["w2"]),
        np.asarray(inputs["b2"]), np.asarray(inputs["w3"]),
        np.asarray(inputs["b3"]),
    )
    W = _build_w(K).astype(ml_dtypes.bfloat16)
    XPb = _build_xpb(x).astype(ml_dtypes.bfloat16)
    Wpre, Wrest = W[:, : FUSE * 128], np.ascontiguousarray(W[:, FUSE * 128 :])
    bias128 = np.ascontiguousarray(
        np.tile(np.asarray(inputs["bias"], dtype=np.float32), 4).reshape(128, 1)
    )

    if "nc" not in _cache:
        _cache["nc"] = _build_nc()
    nc = _cache["nc"]

    n_cores = int(os.environ.get("CK_CORES", str(N_CORES)))
    xw = [
        np.ascontiguousarray(np.concatenate([XPb[b % B], Wpre], axis=1))
        for b in range(n_cores)
    ]
    in_maps = [
        {"xw0": xw[b], "w": Wrest, "bias": bias128} for b in range(n_cores)
    ]

    def unpack(arr):
        return arr.reshape(4, 32, 512).transpose(1, 2, 0).reshape(32, T)

    # The axon-tunneled device occasionally throws a transient
    # NRT_EXEC_UNIT_UNRECOVERABLE on 8-core launches; retry, then fall back
    # to two 4-core waves (same NEFF, batches split across waves).
    res = None
    for attempt in range(3):
        try:
            res = run_bass_kernel_spmd(nc, in_maps, core_ids=list(range(n_cores)))
            break
        except Exception:
            if attempt == 2:
                res = None
            else:
                continue
    if res is not None:
        out = np.stack(
            [unpack(res.results[b % n_cores]["out"]) for b in range(B)], axis=0
        )
        return out.astype(np.float32)

    half = n_cores // 2 if n_cores > 1 else 1
    outs = []
    for w0 in range(0, B, half):
        wave_maps = [
            {"xw0": xw[(w0 + c) % len(xw)], "w": Wrest, "bias": bias128}
            for c in range(half)
        ]
        wres = run_bass_kernel_spmd(nc, wave_maps, core_ids=list(range(half)))
        outs.extend(unpack(wres.results[c]["out"]) for c in range(half))
    out = np.stack(outs[:B], axis=0)
    return out.astype(np.float32)
